# revision 24
# baseline (speedup 1.0000x reference)
"""Trainium2 Bass kernel for nn_Attention_3513283248742.

Bilinear attention: scores = h @ W @ b^T, attn = softmax(scores, -1),
ctx = attn @ b.  Shapes: b [32,1024,1024], h [32,256,1024], W_b [1,1024,1024].

Sharding: data-parallel over batch B=32 across 8 NeuronCores (4 batches per
core); W replicated.  No collectives.

v10 over v2 (110.7us baseline):
  * All 16 hWT groups (4 batches) run FIRST, in greedy availability order
    over (hT batch, W chunk) combos: their DMA deps are just W+hT (4MB,
    four 1MB [Wc_k|hT_k] "ramp" slices) while the 16MB of bT/b streams far
    ahead of the scores/ctx consumers.  This removes v2's ramp stalls
    (scores couldn't start until bT0 landed).
  * Inputs are exactly NINE DMAs on the SP HWDGE ring (4 ramp slices,
    ident, 4x 4MB combined bT+b batch tiles), host-packed to 4-16KB
    contiguous rows (v2's 512B-row descriptors ran the ramp at half rate).
    With <=9 input DMAs, any completion-sem lane reuse by the
    compute-gated out DMAs provably lands on an early-finishing input:
    intermediate revisions lost 14-20us to input/epilogue lane coupling
    (the 8 DMAHW lanes are assigned round-robin in schedule order and a
    reused lane waits out its predecessor).  DMA-xbar transposes for attnT
    were tried and reverted: tile serializes them against every in-flight
    DMA, so they stall behind the whole input stream.
  * The PE warmup runs garbage-fed matmuls gated only on a DVE memset, so
    it starts right after the preamble and trips the HAM activity window
    (cold PE runs at 1.2GHz; warm at 2.4GHz) before the real work.
  * A dummy exp during the ramp preloads the ACT exp table (the first
    activation otherwise pays a 1.3us table load inside softmax0).
  * PE stream after the hWT phase: s0 at0a s1 at0b at1a ctx0a at1b ctx0b
    s2 ... -- each batch's softmax latency hides behind the next batch's
    scores matmuls.
  * ctx PSUM is split per 512-col half; the last batch splits its final
    half into N=256 quarter groups with the out DMAs on the otherwise-idle
    SP ring, so the drain waits only on a 64KB write.

Per-core pipeline (per batch i):
  hWT  = W^T @ hT_i           lhsT = W chunks, rhs = hT (from host)
  S    = hWT^T @ bT_i         scores [q,k]
  softmax over k: DVE row max, ACT exp (+rowsum via accum), DVE recip
  attnT = DMA xbar transpose of E (fp16, SBUF->SBUF)
  ctx  = attnT^T @ b_i        rhs = b natural layout (from host)
  out  = ctx * invS           ACT epilogue, fp16, DMA'd from the ACT queue
"""

import numpy as np

import concourse.bass as bass
import concourse.mybir as mybir
import concourse.tile as tile
from concourse.bass_utils import run_bass_kernel_spmd
from concourse.vector_clock import ScopedClock

F32 = mybir.dt.float32
F16 = mybir.dt.float16

N_CORES = 8
B, TB, TH, D = 32, 1024, 256, 1024
BPC = B // N_CORES  # batches per core = 4
P = 128
NDC = D // P   # 8 chunks of the D axis
NKC = TB // P  # 8 chunks of the k axis
NQ = TH // P   # 2 chunks of the q axis

# flat element offsets inside the ramp tile [P, 16384] (f16):
# [Wc0 | hT0 | Wc1 | hT1 | Wc2 | hT2 | Wc3 | hT3], 2048 elems each.
# Pairing Wc_k with hT_k per 1MB ramp slice maximizes the (W chunk x hT
# batch) work combos unlocked per arriving slice.
WOFF = [0, 4096, 8192, 12288]
HOFF = [2048, 6144, 10240, 14336]
BB_B = NDC * TB  # b-natural part offset inside the bb tile (elems)

_PATCHED = False
CLEAR_SEMS_ON_EXIT = False


def _patch_tile_drain(max_waits_per_inst: int = 1):
    """This walrus build rejects >1 sem wait on the SP Drain instruction that
    TileContext emits on exit; split the waits across preceding sync nops."""
    global _PATCHED
    if _PATCHED:
        return
    _PATCHED = True

    def _drain_and_barrier(self, tick_clock, wait_clock):
        nc = self.nc
        drain_inst = nc.sync.drain()
        wait_clock.add_sem_waits(
            drain_inst.ins, ScopedClock({None: tick_clock.global_clock})
        )
        si = drain_inst.ins.sync_info
        if si is not None and si.on_wait and len(si.on_wait) > max_waits_per_inst:
            waits = list(si.on_wait)
            bb = nc.cur_bb.bb
            assert bb.instructions[-1] is drain_inst.ins
            bb.instructions.pop()
            si.on_wait = waits[:max_waits_per_inst]
            rest = waits[max_waits_per_inst:]
            for i in range(0, len(rest), max_waits_per_inst):
                nop = nc.sync.nop(nofuse=True)
                chunk = rest[i : i + max_waits_per_inst]
                if nop.ins.sync_info is None:
                    nop.ins.sync_info = mybir.SyncInfo(on_wait=chunk, on_update=[])
                else:
                    nop.ins.sync_info.on_wait.extend(chunk)
            bb.instructions.append(drain_inst.ins)
        nc.all_engine_barrier()
        assert self.sems is not None
        popped = nc._tile_sem_poison_stack.pop()
        assert popped is self._sem_poison
        if CLEAR_SEMS_ON_EXIT:
            nc.clear_and_free_semaphores(list(self.sems.allocated().values()))
            nc.all_engine_barrier()
        else:
            nc._state.prepend_free_semaphores(
                [
                    s.num if hasattr(s, "num") else s
                    for s in self.sems.allocated().values()
                ]
            )

    tile.TileContext._drain_and_barrier = _drain_and_barrier


def _split_excess_waits(nc, max_waits: int = 1):
    """Walrus rejects instructions carrying more than `max_waits` sem waits.
    Hoist excess waits onto same-engine nops inserted just before."""
    for f in nc.m.functions:
        for bb in f.blocks:
            out = []
            for ins in list(bb.instructions):
                si = ins.sync_info
                if si is not None and si.on_wait and len(si.on_wait) > max_waits:
                    waits = list(si.on_wait)
                    si.on_wait = waits[:max_waits]
                    rest = waits[max_waits:]
                    for i in range(0, len(rest), max_waits):
                        nop = nc.engines[ins.engine].nop(nofuse=True)
                        cur_bb = nc.cur_bb.bb
                        assert cur_bb.instructions[-1] is nop.ins
                        cur_bb.instructions.pop()
                        nop.ins.sync_info = mybir.SyncInfo(
                            on_wait=rest[i : i + max_waits], on_update=[]
                        )
                        out.append(nop.ins)
                out.append(ins)
            bb.instructions[:] = out


def build_nc():
    _patch_tile_drain()
    nc = bass.Bass(trn_type="TRN2", target_bir_lowering=False, debug=False)
    # bb = per-batch [bT (NDC*TB) | b-natural (NKC*D)] packed per partition
    bb_ext = nc.declare_dram_parameter("bb", [BPC, P, 2 * BB_B], F16, isOutput=False)
    ramp_ext = nc.declare_dram_parameter("ramp", [P, 16384], F16, isOutput=False)
    ident_ext = nc.declare_dram_parameter("ident", [P, P], F16, isOutput=False)
    out_ext = nc.declare_dram_parameter("out", [BPC, TH, D], F16, isOutput=True)

    with tile.TileContext(nc) as tc:
        with (
            tc.tile_pool(name="consts", bufs=1) as consts,
            tc.tile_pool(name="bbpool", bufs=4) as bbpool,
            tc.tile_pool(name="hwtpool", bufs=4) as hwtpool,
            tc.tile_pool(name="epool", bufs=2) as epool,
            tc.tile_pool(name="atpool", bufs=2) as atpool,
            tc.tile_pool(name="ctxpool", bufs=2) as ctxpool,
            tc.tile_pool(name="stats", bufs=2) as stats,
            tc.tile_pool(name="psbig", bufs=2, space="PSUM") as psbig,
            tc.tile_pool(name="pshw", bufs=2, space="PSUM") as pshw,
            tc.tile_pool(name="psT", bufs=2, space="PSUM") as psT,
        ):
            # --- exactly 9 input DMAs, all on the SP HWDGE ring, in
            # priority order.  ident sits after the ramp slices (it is not
            # needed until the first attnT transpose at ~46us).
            ident_t = consts.tile([P, P], F16)
            ramp = consts.tile([P, 16384], F16)  # W chunks + hT batches
            # slice 1 is split at hT0's j0-3 boundary so batch0/chunk0's
            # first four matmuls start 0.25MB (~0.9us) earlier
            bounds = [0, 3072, 4096, 8192, 12288, 16384]
            for k in range(5):
                nc.sync.dma_start(
                    ramp[:, bounds[k] : bounds[k + 1]],
                    ramp_ext.ap()[:, bounds[k] : bounds[k + 1]],
                )
            nc.sync.dma_start(ident_t[:], ident_ext.ap())
            ident16 = ident_t[:]
            bb = [None] * BPC
            for i in range(BPC):
                bb[i] = bbpool.tile([P, 2 * BB_B], F16, name=f"bb{i}", tag="bb")
                nc.sync.dma_start(bb[i][:], bb_ext[i])

            # --- ACT: preload the exp activation table off the critical path
            dummy = stats.tile([P, 4], F16, name="dummy", tag="dummy")
            nc.scalar.activation(
                dummy[:], ident_t[:, 0:4], mybir.ActivationFunctionType.Exp
            )

            # --- PE warmup: garbage-fed N=256 matmuls with NO DMA dep (the
            # scratch tile is memset on DVE), so the PE starts right after
            # the preamble, fills the ~6us wait for the first ramp slice,
            # and trips the HAM activity window so hWT starts at 2.4GHz.
            warm_in = consts.tile([P, 256], F16)
            nc.vector.memset(warm_in[:], 0)
            for wi in range(20):
                wt = pshw.tile([P, 512], F32, name="warm", tag="pshw")
                nc.tensor.matmul(
                    wt[:, 0:256], warm_in[:, 0:P], warm_in[:], start=True, stop=True
                )

            # --- hWT phase: all 16 groups (4 batches x 4 dout-pair groups).
            # Group (i, tp) depends on W chunk tp/2 and hT_i only, so batch
            # 0's groups chase the ramp-slice DMAs.
            hWT = [
                hwtpool.tile([P, NDC, TH], F16, name=f"hWT{i}", tag="hWT")
                for i in range(BPC)
            ]

            def emit_hwt_group(i, tp):
                """One tp-group (2 dout chunks) of hWT for batch i. 16 mm."""
                ps = pshw.tile([P, 512], F32, name="ps_hw", tag="pshw")
                for dt in range(2):
                    t = tp + dt
                    c, half = t // 2, t % 2
                    for j in range(NDC):
                        wb = WOFF[c] + j * 256 + half * P
                        hb = HOFF[i] + j * TH
                        nc.tensor.matmul(
                            ps[:, dt * 256 : (dt + 1) * 256],
                            ramp[:, wb : wb + P],
                            ramp[:, hb : hb + TH],
                            start=(j == 0),
                            stop=(j == NDC - 1),
                        )
                nc.vector.tensor_copy(
                    hWT[i][:, tp : tp + 2, :].rearrange("p a b -> p (a b)"),
                    ps[:],
                )

            # greedy availability order: group (i, c) needs ramp slice
            # max(i, c); emit in the order work is unlocked by the stream.
            for i, c in [
                (0, 0), (0, 1), (1, 0), (1, 1),
                (0, 2), (1, 2), (2, 0), (2, 1), (2, 2),
                (0, 3), (1, 3), (2, 3), (3, 0), (3, 1), (3, 2), (3, 3),
            ]:
                emit_hwt_group(i, 2 * c)

            # --- per-batch stream ---
            def make_batch(i):
                E = epool.tile([P, NQ, TB], F16, name=f"E{i}", tag="E")
                negmax = stats.tile([P, NQ, 1], F32, name="negmax", tag="negmax")
                S_sum = stats.tile([P, NQ, 1], F32, name="S_sum", tag="S")
                invS = stats.tile([P, NQ, 1], F32, name="invS", tag="invS")
                # attnT[p, r, c, q] = E[q, r, c*128+p]: one xbar per batch
                attnT = atpool.tile([P, NQ, NKC, P], F16, name=f"attnT{i}", tag="attnT")
                ctx16 = ctxpool.tile([P, NQ, D], F16, name=f"ctx{i}", tag="ctx")
                ps_scores = [None] * NQ

                def scores_mm(r, kh):
                    if ps_scores[r] is None:
                        ps_scores[r] = psbig.tile([P, TB], F32, name="ps_s", tag="psb")
                    ps_s = ps_scores[r]
                    for j in range(NDC):
                        kb = j * TB + kh * 512
                        nc.tensor.matmul(
                            ps_s[:, kh * 512 : (kh + 1) * 512],
                            hWT[i][:, j, r * P : (r + 1) * P],
                            bb[i][:, kb : kb + 512],
                            start=(j == 0),
                            stop=(j == NDC - 1),
                        )

                def softmax_half(r):
                    # DVE rowmax -> ACT exp (rowsum via accum) -> DVE recip
                    ps_s = ps_scores[r]
                    nc.vector.tensor_reduce(
                        negmax[:, r, :],
                        ps_s[:],
                        axis=mybir.AxisListType.X,
                        op=mybir.AluOpType.max,
                        negate=True,
                    )
                    nc.scalar.activation(
                        E[:, r, :],
                        ps_s[:],
                        mybir.ActivationFunctionType.Exp,
                        bias=negmax[:, r, :],
                        accum_out=S_sum[:, r, :],
                    )
                    nc.vector.reciprocal(invS[:, r, :], S_sum[:, r, :])

                def attnT_half(r):
                    # PE transposes of E (DMA-xbar transpose is unusable
                    # here: tile serializes it against every in-flight DMA,
                    # so it can't run until the 20MB input stream drains)
                    ps = psT.tile([P, TB], F16, name="ps_at", tag="ps16")
                    for c in range(NKC):
                        nc.tensor.transpose(
                            ps[:, c * P : (c + 1) * P],
                            E[:, r, c * P : (c + 1) * P],
                            ident16,
                        )
                    nc.vector.tensor_copy(
                        attnT[:, r].rearrange("p a b -> p (a b)"),
                        ps[:],
                    )

                def ctx_mm(r, split_out=False):
                    # separate [P,512] PSUM tiles per half: the half-0
                    # epilogue (mul reads PSUM) must not carry a
                    # tile-granular WAR against the half-1 matmuls
                    for dh in range(2):
                        quarters = 2 if (split_out and dh == 1) else 1
                        width = 512 // quarters
                        for qq in range(quarters):
                            # fresh PSUM tile per piece: the piece-A mul
                            # (ACT read) must not impose a tile-granular WAR
                            # on piece-B's matmuls
                            ps_h = pshw.tile([P, 512], F32, name="ps_cs", tag="pshw")
                            for c in range(NKC):
                                db = BB_B + c * D + dh * 512 + qq * width
                                nc.tensor.matmul(
                                    ps_h[:, 0:width],
                                    attnT[:, r, c, :],
                                    bb[i][:, db : db + width],
                                    start=(c == 0),
                                    stop=(c == NKC - 1),
                                )
                            sl = slice(
                                dh * 512 + qq * width, dh * 512 + (qq + 1) * width
                            )
                            nc.scalar.mul(
                                ctx16[:, r, sl], ps_h[:, 0:width], invS[:, r, :]
                            )
                            if split_out:
                                # final pieces ride the idle SP ring so the
                                # last write never queues behind earlier outs
                                nc.sync.dma_start(
                                    out_ext[i, r * P : (r + 1) * P, sl],
                                    ctx16[:, r, sl],
                                )
                    if not split_out:
                        nc.scalar.dma_start(
                            out_ext[i, r * P : (r + 1) * P, :], ctx16[:, r, :]
                        )

                return scores_mm, softmax_half, attnT_half, ctx_mm

            # PE stream: per batch, scores -> attnT transposes -> ctx, with
            # the next batch's scores as filler so softmax latency never
            # stalls the PE.
            ops = [make_batch(i) for i in range(BPC)]

            def emit_scores(i):
                scores_mm, softmax_half, _, _ = ops[i]
                scores_mm(0, 0)
                scores_mm(0, 1)
                softmax_half(0)
                scores_mm(1, 0)
                scores_mm(1, 1)
                softmax_half(1)

            def emit_at(i, r):
                ops[i][2](r)

            def emit_ctx_r(i, r, last=False):
                ops[i][3](r, split_out=last)

            emit_scores(0)
            emit_at(0, 0)
            emit_scores(1)
            emit_at(0, 1)
            emit_at(1, 0)
            emit_ctx_r(0, 0)
            emit_at(1, 1)
            emit_ctx_r(0, 1)
            emit_scores(2)
            emit_at(2, 0)
            emit_ctx_r(1, 0)
            emit_at(2, 1)
            emit_ctx_r(1, 1)
            emit_scores(3)
            emit_at(3, 0)
            emit_ctx_r(2, 0)
            emit_at(3, 1)
            emit_ctx_r(2, 1)
            emit_ctx_r(3, 0)
            emit_ctx_r(3, 1, last=True)
    _split_excess_waits(nc)
    return nc


_NC_CACHE = None


def _get_nc():
    global _NC_CACHE
    if _NC_CACHE is None:
        _NC_CACHE = build_nc()
    return _NC_CACHE


def run(b, h, W_b, trace=False):
    """Shard, execute on 8 cores, gather. Returns (ctx, BassKernelResults)."""
    assert b.shape == (B, TB, D) and h.shape == (B, TH, D)
    # All on-chip compute is fp16; cast and pre-pack on the host so every
    # DMA moves >=4KB contiguous per partition and the PE never does layout.
    W16 = W_b[0].astype(np.float16)  # [D, D]
    # W chunk c as [P, NDC*256]: w[c][p, j*256+d] = W[j*128+p, c*256+d]
    wr = W16.reshape(NDC, P, 4, 256).transpose(2, 1, 0, 3).reshape(4, P, NDC * 256)
    h16 = h.astype(np.float16)
    # hT batch i as [P, NDC*TH]: hT[i][p, c*TH+q] = h[i, q, c*128+p]
    hTr = h16.reshape(B, TH, NDC, P).transpose(0, 3, 2, 1).reshape(B, P, NDC * TH)
    b16 = b.astype(np.float16)
    # bT[i][p, c*TB+k] = b[i, k, c*128+p]
    bTr = b16.reshape(B, TB, NDC, P).transpose(0, 3, 2, 1).reshape(B, P, NDC * TB)
    # bn[i][p, c*D+d] = b[i, c*128+p, d]
    bnr = b16.reshape(B, NKC, P, D).transpose(0, 2, 1, 3).reshape(B, P, NKC * D)
    bbr = np.ascontiguousarray(np.concatenate([bTr, bnr], axis=2))  # [B, P, 2*BB_B]
    ramp_parts = [wr[0], hTr[0], wr[1], wr[2], wr[3]]  # per-core hT below
    ident = np.eye(P, dtype=np.float16)
    in_maps = []
    for c in range(N_CORES):
        sl = slice(c * BPC, (c + 1) * BPC)
        i0 = c * BPC
        ramp_np = np.ascontiguousarray(
            np.concatenate(
                [wr[0], hTr[i0], wr[1], hTr[i0 + 1], wr[2], hTr[i0 + 2],
                 wr[3], hTr[i0 + 3]],
                axis=1,
            )
        )
        in_maps.append(
            {
                "bb": bbr[sl],
                "ramp": ramp_np,
                "ident": ident,
            }
        )
    res = run_bass_kernel_spmd(
        _get_nc(), in_maps, core_ids=list(range(N_CORES)), trace=trace
    )
    out = np.concatenate([res.results[c]["out"] for c in range(N_CORES)], axis=0)
    return out.astype(np.float32), res


def kernel(b, h, W_b):
    out, _ = run(b, h, W_b, trace=False)
    return out


# revision 25
# speedup vs baseline: 1.0246x; 1.0246x over previous
"""Trainium2 Bass kernel for nn_Attention_3513283248742.

Bilinear attention: scores = h @ W @ b^T, attn = softmax(scores, -1),
ctx = attn @ b.  Shapes: b [32,1024,1024], h [32,256,1024], W_b [1,1024,1024].

Sharding: data-parallel over batch B=32 across 8 NeuronCores (4 batches per
core); W replicated.  No collectives.

v10 over v2 (110.7us baseline):
  * All 16 hWT groups (4 batches) run FIRST, in greedy availability order
    over (hT batch, W chunk) combos: their DMA deps are just W+hT (4MB,
    four 1MB [Wc_k|hT_k] "ramp" slices) while the 16MB of bT/b streams far
    ahead of the scores/ctx consumers.  This removes v2's ramp stalls
    (scores couldn't start until bT0 landed).
  * Inputs are exactly NINE DMAs on the SP HWDGE ring (4 ramp slices,
    ident, 4x 4MB combined bT+b batch tiles), host-packed to 4-16KB
    contiguous rows (v2's 512B-row descriptors ran the ramp at half rate).
    With <=9 input DMAs, any completion-sem lane reuse by the
    compute-gated out DMAs provably lands on an early-finishing input:
    intermediate revisions lost 14-20us to input/epilogue lane coupling
    (the 8 DMAHW lanes are assigned round-robin in schedule order and a
    reused lane waits out its predecessor).  DMA-xbar transposes for attnT
    were tried and reverted: tile serializes them against every in-flight
    DMA, so they stall behind the whole input stream.
  * The PE warmup runs garbage-fed matmuls gated only on a DVE memset, so
    it starts right after the preamble and trips the HAM activity window
    (cold PE runs at 1.2GHz; warm at 2.4GHz) before the real work.
  * A dummy exp during the ramp preloads the ACT exp table (the first
    activation otherwise pays a 1.3us table load inside softmax0).
  * PE stream after the hWT phase: s0 at0a s1 at0b at1a ctx0a at1b ctx0b
    s2 ... -- each batch's softmax latency hides behind the next batch's
    scores matmuls.
  * ctx PSUM is split per 512-col half; the last batch splits its final
    half into N=256 quarter groups with the out DMAs on the otherwise-idle
    SP ring, so the drain waits only on a 64KB write.

Per-core pipeline (per batch i):
  hWT  = W^T @ hT_i           lhsT = W chunks, rhs = hT (from host)
  S    = hWT^T @ bT_i         scores [q,k]
  softmax over k: DVE row max, ACT exp (+rowsum via accum), DVE recip
  attnT = DMA xbar transpose of E (fp16, SBUF->SBUF)
  ctx  = attnT^T @ b_i        rhs = b natural layout (from host)
  out  = ctx * invS           ACT epilogue, fp16, DMA'd from the ACT queue
"""

import numpy as np

import concourse.bass as bass
import concourse.mybir as mybir
import concourse.tile as tile
from concourse.bass_utils import run_bass_kernel_spmd
from concourse.vector_clock import ScopedClock

F32 = mybir.dt.float32
F16 = mybir.dt.float16

N_CORES = 8
B, TB, TH, D = 32, 1024, 256, 1024
BPC = B // N_CORES  # batches per core = 4
P = 128
NDC = D // P   # 8 chunks of the D axis
NKC = TB // P  # 8 chunks of the k axis
NQ = TH // P   # 2 chunks of the q axis

# flat element offsets inside the ramp tile [P, 16384] (f16):
# [Wc0 | hT0 | Wc1 | hT1 | Wc2 | hT2 | Wc3 | hT3], 2048 elems each.
# Pairing Wc_k with hT_k per 1MB ramp slice maximizes the (W chunk x hT
# batch) work combos unlocked per arriving slice.
WOFF = [0, 4096, 8192, 12288]
HOFF = [2048, 6144, 10240, 14336]
BB_B = NDC * TB  # b-natural part offset inside the bb tile (elems)

_PATCHED = False
CLEAR_SEMS_ON_EXIT = False


def _patch_tile_drain(max_waits_per_inst: int = 1):
    """This walrus build rejects >1 sem wait on the SP Drain instruction that
    TileContext emits on exit; split the waits across preceding sync nops."""
    global _PATCHED
    if _PATCHED:
        return
    _PATCHED = True

    def _drain_and_barrier(self, tick_clock, wait_clock):
        nc = self.nc
        drain_inst = nc.sync.drain()
        wait_clock.add_sem_waits(
            drain_inst.ins, ScopedClock({None: tick_clock.global_clock})
        )
        si = drain_inst.ins.sync_info
        if si is not None and si.on_wait and len(si.on_wait) > max_waits_per_inst:
            waits = list(si.on_wait)
            bb = nc.cur_bb.bb
            assert bb.instructions[-1] is drain_inst.ins
            bb.instructions.pop()
            si.on_wait = waits[:max_waits_per_inst]
            rest = waits[max_waits_per_inst:]
            for i in range(0, len(rest), max_waits_per_inst):
                nop = nc.sync.nop(nofuse=True)
                chunk = rest[i : i + max_waits_per_inst]
                if nop.ins.sync_info is None:
                    nop.ins.sync_info = mybir.SyncInfo(on_wait=chunk, on_update=[])
                else:
                    nop.ins.sync_info.on_wait.extend(chunk)
            bb.instructions.append(drain_inst.ins)
        nc.all_engine_barrier()
        assert self.sems is not None
        popped = nc._tile_sem_poison_stack.pop()
        assert popped is self._sem_poison
        if CLEAR_SEMS_ON_EXIT:
            nc.clear_and_free_semaphores(list(self.sems.allocated().values()))
            nc.all_engine_barrier()
        else:
            nc._state.prepend_free_semaphores(
                [
                    s.num if hasattr(s, "num") else s
                    for s in self.sems.allocated().values()
                ]
            )

    tile.TileContext._drain_and_barrier = _drain_and_barrier


def _split_excess_waits(nc, max_waits: int = 1):
    """Walrus rejects instructions carrying more than `max_waits` sem waits.
    Hoist excess waits onto same-engine nops inserted just before."""
    for f in nc.m.functions:
        for bb in f.blocks:
            out = []
            for ins in list(bb.instructions):
                si = ins.sync_info
                if si is not None and si.on_wait and len(si.on_wait) > max_waits:
                    waits = list(si.on_wait)
                    si.on_wait = waits[:max_waits]
                    rest = waits[max_waits:]
                    for i in range(0, len(rest), max_waits):
                        nop = nc.engines[ins.engine].nop(nofuse=True)
                        cur_bb = nc.cur_bb.bb
                        assert cur_bb.instructions[-1] is nop.ins
                        cur_bb.instructions.pop()
                        nop.ins.sync_info = mybir.SyncInfo(
                            on_wait=rest[i : i + max_waits], on_update=[]
                        )
                        out.append(nop.ins)
                out.append(ins)
            bb.instructions[:] = out


def build_nc():
    _patch_tile_drain()
    nc = bass.Bass(trn_type="TRN2", target_bir_lowering=False, debug=False)
    # bb = per-batch [bT (NDC*TB) | b-natural (NKC*D)] packed per partition
    bb_ext = nc.declare_dram_parameter("bb", [BPC, P, 2 * BB_B], F16, isOutput=False)
    ramp_ext = nc.declare_dram_parameter("ramp", [P, 16384], F16, isOutput=False)
    ident_ext = nc.declare_dram_parameter("ident", [P, P], F16, isOutput=False)
    out_ext = nc.declare_dram_parameter("out", [BPC, TH, D], F16, isOutput=True)

    with tile.TileContext(nc) as tc:
        with (
            tc.tile_pool(name="consts", bufs=1) as consts,
            tc.tile_pool(name="bbpool", bufs=4) as bbpool,
            tc.tile_pool(name="hwtpool", bufs=4) as hwtpool,
            tc.tile_pool(name="epool", bufs=2) as epool,
            tc.tile_pool(name="atpool", bufs=2) as atpool,
            tc.tile_pool(name="ctxpool", bufs=2) as ctxpool,
            tc.tile_pool(name="stats", bufs=2) as stats,
            tc.tile_pool(name="psbig", bufs=2, space="PSUM") as psbig,
            tc.tile_pool(name="pshw", bufs=2, space="PSUM") as pshw,
            tc.tile_pool(name="psT", bufs=2, space="PSUM") as psT,
        ):
            # --- exactly 9 input DMAs, all on the SP HWDGE ring, in
            # priority order.  ident sits after the ramp slices (it is not
            # needed until the first attnT transpose at ~46us).
            ident_t = consts.tile([P, P], F16)
            ramp = consts.tile([P, 16384], F16)  # W chunks + hT batches
            for k in range(4):
                nc.sync.dma_start(
                    ramp[:, k * 4096 : (k + 1) * 4096],
                    ramp_ext.ap()[:, k * 4096 : (k + 1) * 4096],
                )
            nc.sync.dma_start(ident_t[:], ident_ext.ap())
            ident16 = ident_t[:]
            bb = [None] * BPC
            for i in range(BPC):
                bb[i] = bbpool.tile([P, 2 * BB_B], F16, name=f"bb{i}", tag="bb")
                nc.sync.dma_start(bb[i][:], bb_ext[i])

            # --- ACT: preload the exp activation table off the critical path
            dummy = stats.tile([P, 4], F16, name="dummy", tag="dummy")
            nc.scalar.activation(
                dummy[:], ident_t[:, 0:4], mybir.ActivationFunctionType.Exp
            )

            # --- PE warmup: garbage-fed N=256 matmuls with NO DMA dep (the
            # scratch tile is memset on DVE), so the PE starts right after
            # the preamble, fills the ~6us wait for the first ramp slice,
            # and trips the HAM activity window so hWT starts at 2.4GHz.
            warm_in = consts.tile([P, 256], F16)
            nc.vector.memset(warm_in[:], 0)
            for wi in range(20):
                wt = pshw.tile([P, 512], F32, name="warm", tag="pshw")
                nc.tensor.matmul(
                    wt[:, 0:256], warm_in[:, 0:P], warm_in[:], start=True, stop=True
                )

            # --- hWT phase: all 16 groups (4 batches x 4 dout-pair groups).
            # Group (i, tp) depends on W chunk tp/2 and hT_i only, so batch
            # 0's groups chase the ramp-slice DMAs.
            hWT = [
                hwtpool.tile([P, NDC, TH], F16, name=f"hWT{i}", tag="hWT")
                for i in range(BPC)
            ]

            def emit_hwt_group(i, tp):
                """One tp-group (2 dout chunks) of hWT for batch i. 16 mm."""
                ps = pshw.tile([P, 512], F32, name="ps_hw", tag="pshw")
                for dt in range(2):
                    t = tp + dt
                    c, half = t // 2, t % 2
                    for j in range(NDC):
                        wb = WOFF[c] + j * 256 + half * P
                        hb = HOFF[i] + j * TH
                        nc.tensor.matmul(
                            ps[:, dt * 256 : (dt + 1) * 256],
                            ramp[:, wb : wb + P],
                            ramp[:, hb : hb + TH],
                            start=(j == 0),
                            stop=(j == NDC - 1),
                        )
                nc.vector.tensor_copy(
                    hWT[i][:, tp : tp + 2, :].rearrange("p a b -> p (a b)"),
                    ps[:],
                )

            # greedy availability order: group (i, c) needs ramp slice
            # max(i, c); emit in the order work is unlocked by the stream.
            for i, c in [
                (0, 0), (0, 1), (1, 0), (1, 1),
                (0, 2), (1, 2), (2, 0), (2, 1), (2, 2),
                (0, 3), (1, 3), (2, 3), (3, 0), (3, 1), (3, 2), (3, 3),
            ]:
                emit_hwt_group(i, 2 * c)

            # --- per-batch stream ---
            def make_batch(i):
                E = epool.tile([P, NQ, TB], F16, name=f"E{i}", tag="E")
                negmax = stats.tile([P, NQ, 1], F32, name="negmax", tag="negmax")
                S_sum = stats.tile([P, NQ, 1], F32, name="S_sum", tag="S")
                invS = stats.tile([P, NQ, 1], F32, name="invS", tag="invS")
                # attnT[p, r, c, q] = E[q, r, c*128+p]: one xbar per batch
                attnT = atpool.tile([P, NQ, NKC, P], F16, name=f"attnT{i}", tag="attnT")
                ctx16 = ctxpool.tile([P, NQ, D], F16, name=f"ctx{i}", tag="ctx")
                ps_scores = [None] * NQ

                def scores_mm(r, kh):
                    if ps_scores[r] is None:
                        ps_scores[r] = psbig.tile([P, TB], F32, name="ps_s", tag="psb")
                    ps_s = ps_scores[r]
                    for j in range(NDC):
                        kb = j * TB + kh * 512
                        nc.tensor.matmul(
                            ps_s[:, kh * 512 : (kh + 1) * 512],
                            hWT[i][:, j, r * P : (r + 1) * P],
                            bb[i][:, kb : kb + 512],
                            start=(j == 0),
                            stop=(j == NDC - 1),
                        )

                def softmax_half(r):
                    # DVE rowmax -> ACT exp (rowsum via accum) -> DVE recip
                    ps_s = ps_scores[r]
                    nc.vector.tensor_reduce(
                        negmax[:, r, :],
                        ps_s[:],
                        axis=mybir.AxisListType.X,
                        op=mybir.AluOpType.max,
                        negate=True,
                    )
                    nc.scalar.activation(
                        E[:, r, :],
                        ps_s[:],
                        mybir.ActivationFunctionType.Exp,
                        bias=negmax[:, r, :],
                        accum_out=S_sum[:, r, :],
                    )
                    nc.vector.reciprocal(invS[:, r, :], S_sum[:, r, :])

                def attnT_half(r):
                    # PE transposes of E (DMA-xbar transpose is unusable
                    # here: tile serializes it against every in-flight DMA,
                    # so it can't run until the 20MB input stream drains)
                    ps = psT.tile([P, TB], F16, name="ps_at", tag="ps16")
                    for c in range(NKC):
                        nc.tensor.transpose(
                            ps[:, c * P : (c + 1) * P],
                            E[:, r, c * P : (c + 1) * P],
                            ident16,
                        )
                    nc.vector.tensor_copy(
                        attnT[:, r].rearrange("p a b -> p (a b)"),
                        ps[:],
                    )

                def ctx_mm(r, split_out=False):
                    # separate [P,512] PSUM tiles per half: the half-0
                    # epilogue (mul reads PSUM) must not carry a
                    # tile-granular WAR against the half-1 matmuls
                    for dh in range(2):
                        quarters = 2 if (split_out and dh == 1) else 1
                        width = 512 // quarters
                        for qq in range(quarters):
                            # fresh PSUM tile per piece: the piece-A mul
                            # (ACT read) must not impose a tile-granular WAR
                            # on piece-B's matmuls
                            ps_h = pshw.tile([P, 512], F32, name="ps_cs", tag="pshw")
                            for c in range(NKC):
                                db = BB_B + c * D + dh * 512 + qq * width
                                nc.tensor.matmul(
                                    ps_h[:, 0:width],
                                    attnT[:, r, c, :],
                                    bb[i][:, db : db + width],
                                    start=(c == 0),
                                    stop=(c == NKC - 1),
                                )
                            sl = slice(
                                dh * 512 + qq * width, dh * 512 + (qq + 1) * width
                            )
                            nc.scalar.mul(
                                ctx16[:, r, sl], ps_h[:, 0:width], invS[:, r, :]
                            )
                            if split_out:
                                # final pieces ride the idle SP ring so the
                                # last write never queues behind earlier outs
                                nc.sync.dma_start(
                                    out_ext[i, r * P : (r + 1) * P, sl],
                                    ctx16[:, r, sl],
                                )
                    if not split_out:
                        nc.scalar.dma_start(
                            out_ext[i, r * P : (r + 1) * P, :], ctx16[:, r, :]
                        )

                return scores_mm, softmax_half, attnT_half, ctx_mm

            # PE stream: per batch, scores -> attnT transposes -> ctx, with
            # the next batch's scores as filler so softmax latency never
            # stalls the PE.
            ops = [make_batch(i) for i in range(BPC)]

            def emit_scores(i):
                scores_mm, softmax_half, _, _ = ops[i]
                scores_mm(0, 0)
                scores_mm(0, 1)
                softmax_half(0)
                scores_mm(1, 0)
                scores_mm(1, 1)
                softmax_half(1)

            def emit_at(i, r):
                ops[i][2](r)

            def emit_ctx_r(i, r, last=False):
                ops[i][3](r, split_out=last)

            emit_scores(0)
            emit_at(0, 0)
            emit_scores(1)
            emit_at(0, 1)
            emit_at(1, 0)
            emit_ctx_r(0, 0)
            emit_at(1, 1)
            emit_ctx_r(0, 1)
            emit_scores(2)
            emit_at(2, 0)
            emit_ctx_r(1, 0)
            emit_at(2, 1)
            emit_ctx_r(1, 1)
            emit_scores(3)
            emit_at(3, 0)
            emit_ctx_r(2, 0)
            emit_at(3, 1)
            emit_ctx_r(2, 1)
            emit_ctx_r(3, 0)
            emit_ctx_r(3, 1, last=True)
    _split_excess_waits(nc)
    return nc


_NC_CACHE = None


def _get_nc():
    global _NC_CACHE
    if _NC_CACHE is None:
        _NC_CACHE = build_nc()
    return _NC_CACHE


def run(b, h, W_b, trace=False):
    """Shard, execute on 8 cores, gather. Returns (ctx, BassKernelResults)."""
    assert b.shape == (B, TB, D) and h.shape == (B, TH, D)
    # All on-chip compute is fp16; cast and pre-pack on the host so every
    # DMA moves >=4KB contiguous per partition and the PE never does layout.
    W16 = W_b[0].astype(np.float16)  # [D, D]
    # W chunk c as [P, NDC*256]: w[c][p, j*256+d] = W[j*128+p, c*256+d]
    wr = W16.reshape(NDC, P, 4, 256).transpose(2, 1, 0, 3).reshape(4, P, NDC * 256)
    h16 = h.astype(np.float16)
    # hT batch i as [P, NDC*TH]: hT[i][p, c*TH+q] = h[i, q, c*128+p]
    hTr = h16.reshape(B, TH, NDC, P).transpose(0, 3, 2, 1).reshape(B, P, NDC * TH)
    b16 = b.astype(np.float16)
    # bT[i][p, c*TB+k] = b[i, k, c*128+p]
    bTr = b16.reshape(B, TB, NDC, P).transpose(0, 3, 2, 1).reshape(B, P, NDC * TB)
    # bn[i][p, c*D+d] = b[i, c*128+p, d]
    bnr = b16.reshape(B, NKC, P, D).transpose(0, 2, 1, 3).reshape(B, P, NKC * D)
    bbr = np.ascontiguousarray(np.concatenate([bTr, bnr], axis=2))  # [B, P, 2*BB_B]
    ramp_parts = [wr[0], hTr[0], wr[1], wr[2], wr[3]]  # per-core hT below
    ident = np.eye(P, dtype=np.float16)
    in_maps = []
    for c in range(N_CORES):
        sl = slice(c * BPC, (c + 1) * BPC)
        i0 = c * BPC
        ramp_np = np.ascontiguousarray(
            np.concatenate(
                [wr[0], hTr[i0], wr[1], hTr[i0 + 1], wr[2], hTr[i0 + 2],
                 wr[3], hTr[i0 + 3]],
                axis=1,
            )
        )
        in_maps.append(
            {
                "bb": bbr[sl],
                "ramp": ramp_np,
                "ident": ident,
            }
        )
    res = run_bass_kernel_spmd(
        _get_nc(), in_maps, core_ids=list(range(N_CORES)), trace=trace
    )
    out = np.concatenate([res.results[c]["out"] for c in range(N_CORES)], axis=0)
    return out.astype(np.float32), res


def kernel(b, h, W_b):
    out, _ = run(b, h, W_b, trace=False)
    return out


# revision 26
# speedup vs baseline: 1.0270x; 1.0024x over previous
"""Trainium2 Bass kernel for nn_Attention_3513283248742.

Bilinear attention: scores = h @ W @ b^T, attn = softmax(scores, -1),
ctx = attn @ b.  Shapes: b [32,1024,1024], h [32,256,1024], W_b [1,1024,1024].

Sharding: data-parallel over batch B=32 across 8 NeuronCores (4 batches per
core); W replicated.  No collectives.

v13 (104.8us) over v2 (110.7us baseline); measured at the 2.4GHz PE state
(the chip sometimes sits in a ~2.0GHz power state, inflating any
measurement ~19%):
  * All 16 hWT groups (4 batches) run FIRST, in greedy availability order
    over (hT batch, W chunk) combos: their DMA deps are just W+hT (4MB,
    four 1MB [Wc_k|hT_k] "ramp" slices) while the 16MB of bT/b streams far
    ahead of the scores/ctx consumers.  This removes v2's ramp stalls
    (scores couldn't start until bT0 landed).
  * Inputs are exactly NINE DMAs on the SP HWDGE ring (4 ramp slices,
    ident, 4x 4MB combined bT+b batch tiles), host-packed to 4-16KB
    contiguous rows (v2's 512B-row descriptors ran the ramp at half rate).
    With <=9 input DMAs, any completion-sem lane reuse by the
    compute-gated out DMAs provably lands on an early-finishing input:
    intermediate revisions lost 14-20us to input/epilogue lane coupling
    (the 8 DMAHW lanes are assigned round-robin in schedule order and a
    reused lane waits out its predecessor).  DMA-xbar transposes for attnT
    were tried and reverted: tile serializes them against every in-flight
    DMA, so they stall behind the whole input stream.
  * The PE warmup runs garbage-fed matmuls gated only on a DVE memset, so
    it starts right after the preamble and trips the HAM activity window
    (cold PE runs at 1.2GHz; warm at 2.4GHz) before the real work.
  * A dummy exp during the ramp preloads the ACT exp table (the first
    activation otherwise pays a 1.3us table load inside softmax0).
  * PE stream after the hWT phase: s0 at0a s1 at0b at1a ctx0a at1b ctx0b
    s2 ... -- each batch's softmax latency hides behind the next batch's
    scores matmuls.
  * ctx PSUM is split per 512-col half; the last batch splits its final
    half into N=256 quarter groups with the out DMAs on the otherwise-idle
    SP ring, so the drain waits only on a 64KB write.

Per-core pipeline (per batch i):
  hWT  = W^T @ hT_i           lhsT = W chunks, rhs = hT (from host)
  S    = hWT^T @ bT_i         scores [q,k]
  softmax over k: DVE row max, ACT exp (+rowsum via accum), DVE recip
  attnT = DMA xbar transpose of E (fp16, SBUF->SBUF)
  ctx  = attnT^T @ b_i        rhs = b natural layout (from host)
  out  = ctx * invS           ACT epilogue, fp16, DMA'd from the ACT queue
"""

import numpy as np

import concourse.bass as bass
import concourse.mybir as mybir
import concourse.tile as tile
from concourse.bass_utils import run_bass_kernel_spmd
from concourse.vector_clock import ScopedClock

F32 = mybir.dt.float32
F16 = mybir.dt.float16

N_CORES = 8
B, TB, TH, D = 32, 1024, 256, 1024
BPC = B // N_CORES  # batches per core = 4
P = 128
NDC = D // P   # 8 chunks of the D axis
NKC = TB // P  # 8 chunks of the k axis
NQ = TH // P   # 2 chunks of the q axis

# flat element offsets inside the ramp tile [P, 16384] (f16):
# [Wc0 | hT0 | Wc1 | hT1 | Wc2 | hT2 | Wc3 | hT3], 2048 elems each.
# Pairing Wc_k with hT_k per 1MB ramp slice maximizes the (W chunk x hT
# batch) work combos unlocked per arriving slice.
WOFF = [0, 4096, 8192, 12288]
HOFF = [2048, 6144, 10240, 14336]
BB_B = NDC * TB  # b-natural part offset inside the bb tile (elems)

_PATCHED = False
CLEAR_SEMS_ON_EXIT = False


def _patch_tile_drain(max_waits_per_inst: int = 1):
    """This walrus build rejects >1 sem wait on the SP Drain instruction that
    TileContext emits on exit; split the waits across preceding sync nops."""
    global _PATCHED
    if _PATCHED:
        return
    _PATCHED = True

    def _drain_and_barrier(self, tick_clock, wait_clock):
        nc = self.nc
        drain_inst = nc.sync.drain()
        wait_clock.add_sem_waits(
            drain_inst.ins, ScopedClock({None: tick_clock.global_clock})
        )
        si = drain_inst.ins.sync_info
        if si is not None and si.on_wait and len(si.on_wait) > max_waits_per_inst:
            waits = list(si.on_wait)
            bb = nc.cur_bb.bb
            assert bb.instructions[-1] is drain_inst.ins
            bb.instructions.pop()
            si.on_wait = waits[:max_waits_per_inst]
            rest = waits[max_waits_per_inst:]
            for i in range(0, len(rest), max_waits_per_inst):
                nop = nc.sync.nop(nofuse=True)
                chunk = rest[i : i + max_waits_per_inst]
                if nop.ins.sync_info is None:
                    nop.ins.sync_info = mybir.SyncInfo(on_wait=chunk, on_update=[])
                else:
                    nop.ins.sync_info.on_wait.extend(chunk)
            bb.instructions.append(drain_inst.ins)
        nc.all_engine_barrier()
        assert self.sems is not None
        popped = nc._tile_sem_poison_stack.pop()
        assert popped is self._sem_poison
        if CLEAR_SEMS_ON_EXIT:
            nc.clear_and_free_semaphores(list(self.sems.allocated().values()))
            nc.all_engine_barrier()
        else:
            nc._state.prepend_free_semaphores(
                [
                    s.num if hasattr(s, "num") else s
                    for s in self.sems.allocated().values()
                ]
            )

    tile.TileContext._drain_and_barrier = _drain_and_barrier


def _split_excess_waits(nc, max_waits: int = 1):
    """Walrus rejects instructions carrying more than `max_waits` sem waits.
    Hoist excess waits onto same-engine nops inserted just before."""
    for f in nc.m.functions:
        for bb in f.blocks:
            out = []
            for ins in list(bb.instructions):
                si = ins.sync_info
                if si is not None and si.on_wait and len(si.on_wait) > max_waits:
                    waits = list(si.on_wait)
                    si.on_wait = waits[:max_waits]
                    rest = waits[max_waits:]
                    for i in range(0, len(rest), max_waits):
                        nop = nc.engines[ins.engine].nop(nofuse=True)
                        cur_bb = nc.cur_bb.bb
                        assert cur_bb.instructions[-1] is nop.ins
                        cur_bb.instructions.pop()
                        nop.ins.sync_info = mybir.SyncInfo(
                            on_wait=rest[i : i + max_waits], on_update=[]
                        )
                        out.append(nop.ins)
                out.append(ins)
            bb.instructions[:] = out


def build_nc():
    _patch_tile_drain()
    nc = bass.Bass(trn_type="TRN2", target_bir_lowering=False, debug=False)
    # bb = per-batch [bT (NDC*TB) | b-natural (NKC*D)] packed per partition
    bb_ext = nc.declare_dram_parameter("bb", [BPC, P, 2 * BB_B], F16, isOutput=False)
    ramp_ext = nc.declare_dram_parameter("ramp", [P, 16384], F16, isOutput=False)
    ident_ext = nc.declare_dram_parameter("ident", [P, P], F16, isOutput=False)
    out_ext = nc.declare_dram_parameter("out", [BPC, TH, D], F16, isOutput=True)

    with tile.TileContext(nc) as tc:
        with (
            tc.tile_pool(name="consts", bufs=1) as consts,
            tc.tile_pool(name="bbpool", bufs=4) as bbpool,
            tc.tile_pool(name="hwtpool", bufs=4) as hwtpool,
            tc.tile_pool(name="epool", bufs=2) as epool,
            tc.tile_pool(name="atpool", bufs=2) as atpool,
            tc.tile_pool(name="ctxpool", bufs=2) as ctxpool,
            tc.tile_pool(name="stats", bufs=2) as stats,
            tc.tile_pool(name="psbig", bufs=2, space="PSUM") as psbig,
            tc.tile_pool(name="pshw", bufs=2, space="PSUM") as pshw,
            tc.tile_pool(name="psT", bufs=2, space="PSUM") as psT,
        ):
            # --- exactly 9 input DMAs, all on the SP HWDGE ring, in
            # priority order.  ident sits after the ramp slices (it is not
            # needed until the first attnT transpose at ~46us).
            ident_t = consts.tile([P, P], F16)
            ramp = consts.tile([P, 16384], F16)  # W chunks + hT batches
            for k in range(4):
                nc.sync.dma_start(
                    ramp[:, k * 4096 : (k + 1) * 4096],
                    ramp_ext.ap()[:, k * 4096 : (k + 1) * 4096],
                )
            nc.sync.dma_start(ident_t[:], ident_ext.ap())
            ident16 = ident_t[:]
            bb = [None] * BPC
            for i in range(BPC):
                bb[i] = bbpool.tile([P, 2 * BB_B], F16, name=f"bb{i}", tag="bb")
                nc.sync.dma_start(bb[i][:], bb_ext[i])

            # --- ACT: preload the exp activation table off the critical path
            dummy = stats.tile([P, 4], F16, name="dummy", tag="dummy")
            nc.scalar.activation(
                dummy[:], ident_t[:, 0:4], mybir.ActivationFunctionType.Exp
            )

            # --- PE warmup: garbage-fed N=256 matmuls with NO DMA dep (the
            # scratch tile is memset on DVE), so the PE starts right after
            # the preamble, fills the ~6us wait for the first ramp slice,
            # and trips the HAM activity window so hWT starts at 2.4GHz.
            warm_in = consts.tile([P, 256], F16)
            nc.vector.memset(warm_in[:], 0)
            for wi in range(20):
                wt = pshw.tile([P, 512], F32, name="warm", tag="pshw")
                nc.tensor.matmul(
                    wt[:, 0:256], warm_in[:, 0:P], warm_in[:], start=True, stop=True
                )

            # --- hWT phase: all 16 groups (4 batches x 4 dout-pair groups).
            # Group (i, tp) depends on W chunk tp/2 and hT_i only, so batch
            # 0's groups chase the ramp-slice DMAs.
            hWT = [
                hwtpool.tile([P, NDC, TH], F16, name=f"hWT{i}", tag="hWT")
                for i in range(BPC)
            ]

            def emit_hwt_group(i, tp):
                """One tp-group (2 dout chunks) of hWT for batch i. 16 mm."""
                ps = pshw.tile([P, 512], F32, name="ps_hw", tag="pshw")
                for dt in range(2):
                    t = tp + dt
                    c, half = t // 2, t % 2
                    for j in range(NDC):
                        wb = WOFF[c] + j * 256 + half * P
                        hb = HOFF[i] + j * TH
                        nc.tensor.matmul(
                            ps[:, dt * 256 : (dt + 1) * 256],
                            ramp[:, wb : wb + P],
                            ramp[:, hb : hb + TH],
                            start=(j == 0),
                            stop=(j == NDC - 1),
                        )
                nc.vector.tensor_copy(
                    hWT[i][:, tp : tp + 2, :].rearrange("p a b -> p (a b)"),
                    ps[:],
                )

            # greedy availability order: group (i, c) needs ramp slice
            # max(i, c); emit in the order work is unlocked by the stream.
            for i, c in [
                (0, 0), (0, 1), (1, 0), (1, 1),
                (0, 2), (1, 2), (2, 0), (2, 1), (2, 2),
                (0, 3), (1, 3), (2, 3), (3, 0), (3, 1), (3, 2), (3, 3),
            ]:
                emit_hwt_group(i, 2 * c)

            # --- per-batch stream ---
            def make_batch(i):
                E = epool.tile([P, NQ, TB], F16, name=f"E{i}", tag="E")
                negmax = stats.tile([P, NQ, 1], F32, name="negmax", tag="negmax")
                S_sum = stats.tile([P, NQ, 1], F32, name="S_sum", tag="S")
                invS = stats.tile([P, NQ, 1], F32, name="invS", tag="invS")
                # attnT[p, r, c, q] = E[q, r, c*128+p]: one xbar per batch
                attnT = atpool.tile([P, NQ, NKC, P], F16, name=f"attnT{i}", tag="attnT")
                ctx16 = ctxpool.tile([P, NQ, D], F16, name=f"ctx{i}", tag="ctx")
                ps_scores = [None] * NQ

                def scores_mm(r, kh):
                    if ps_scores[r] is None:
                        ps_scores[r] = psbig.tile([P, TB], F32, name="ps_s", tag="psb")
                    ps_s = ps_scores[r]
                    for j in range(NDC):
                        kb = j * TB + kh * 512
                        nc.tensor.matmul(
                            ps_s[:, kh * 512 : (kh + 1) * 512],
                            hWT[i][:, j, r * P : (r + 1) * P],
                            bb[i][:, kb : kb + 512],
                            start=(j == 0),
                            stop=(j == NDC - 1),
                        )

                def softmax_half(r):
                    # DVE rowmax -> ACT exp (rowsum via accum) -> DVE recip
                    ps_s = ps_scores[r]
                    nc.vector.tensor_reduce(
                        negmax[:, r, :],
                        ps_s[:],
                        axis=mybir.AxisListType.X,
                        op=mybir.AluOpType.max,
                        negate=True,
                    )
                    nc.scalar.activation(
                        E[:, r, :],
                        ps_s[:],
                        mybir.ActivationFunctionType.Exp,
                        bias=negmax[:, r, :],
                        accum_out=S_sum[:, r, :],
                    )
                    nc.vector.reciprocal(invS[:, r, :], S_sum[:, r, :])

                def attnT_half(r):
                    # PE transposes of E (DMA-xbar transpose is unusable
                    # here: tile serializes it against every in-flight DMA,
                    # so it can't run until the 20MB input stream drains)
                    ps = psT.tile([P, TB], F16, name="ps_at", tag="ps16")
                    for c in range(NKC):
                        nc.tensor.transpose(
                            ps[:, c * P : (c + 1) * P],
                            E[:, r, c * P : (c + 1) * P],
                            ident16,
                        )
                    nc.vector.tensor_copy(
                        attnT[:, r].rearrange("p a b -> p (a b)"),
                        ps[:],
                    )

                def ctx_mm(r, split_out=False):
                    # separate [P,512] PSUM tiles per half: the half-0
                    # epilogue (mul reads PSUM) must not carry a
                    # tile-granular WAR against the half-1 matmuls
                    for dh in range(2):
                        quarters = 2 if (split_out and dh == 1) else 1
                        width = 512 // quarters
                        for qq in range(quarters):
                            # fresh PSUM tile per piece: the piece-A mul
                            # (ACT read) must not impose a tile-granular WAR
                            # on piece-B's matmuls
                            ps_h = pshw.tile([P, 512], F32, name="ps_cs", tag="pshw")
                            for c in range(NKC):
                                db = BB_B + c * D + dh * 512 + qq * width
                                nc.tensor.matmul(
                                    ps_h[:, 0:width],
                                    attnT[:, r, c, :],
                                    bb[i][:, db : db + width],
                                    start=(c == 0),
                                    stop=(c == NKC - 1),
                                )
                            sl = slice(
                                dh * 512 + qq * width, dh * 512 + (qq + 1) * width
                            )
                            nc.scalar.mul(
                                ctx16[:, r, sl], ps_h[:, 0:width], invS[:, r, :]
                            )
                            if split_out:
                                # final pieces ride the idle SP ring so the
                                # last write never queues behind earlier outs
                                nc.sync.dma_start(
                                    out_ext[i, r * P : (r + 1) * P, sl],
                                    ctx16[:, r, sl],
                                )
                    if not split_out:
                        nc.scalar.dma_start(
                            out_ext[i, r * P : (r + 1) * P, :], ctx16[:, r, :]
                        )

                return scores_mm, softmax_half, attnT_half, ctx_mm

            # PE stream: per batch, scores -> attnT transposes -> ctx, with
            # the next batch's scores as filler so softmax latency never
            # stalls the PE.
            ops = [make_batch(i) for i in range(BPC)]

            def emit_scores(i):
                scores_mm, softmax_half, _, _ = ops[i]
                scores_mm(0, 0)
                scores_mm(0, 1)
                softmax_half(0)
                scores_mm(1, 0)
                scores_mm(1, 1)
                softmax_half(1)

            def emit_at(i, r):
                ops[i][2](r)

            def emit_ctx_r(i, r, last=False):
                ops[i][3](r, split_out=last)

            emit_scores(0)
            emit_at(0, 0)
            emit_scores(1)
            emit_at(0, 1)
            emit_at(1, 0)
            emit_ctx_r(0, 0)
            emit_at(1, 1)
            emit_ctx_r(0, 1)
            emit_scores(2)
            emit_at(2, 0)
            emit_ctx_r(1, 0)
            emit_at(2, 1)
            emit_ctx_r(1, 1)
            emit_scores(3)
            emit_at(3, 0)
            emit_ctx_r(2, 0)
            emit_at(3, 1)
            emit_ctx_r(2, 1)
            emit_ctx_r(3, 0)
            emit_ctx_r(3, 1, last=True)
    _split_excess_waits(nc)
    return nc


_NC_CACHE = None


def _get_nc():
    global _NC_CACHE
    if _NC_CACHE is None:
        _NC_CACHE = build_nc()
    return _NC_CACHE


def run(b, h, W_b, trace=False):
    """Shard, execute on 8 cores, gather. Returns (ctx, BassKernelResults)."""
    assert b.shape == (B, TB, D) and h.shape == (B, TH, D)
    # All on-chip compute is fp16; cast and pre-pack on the host so every
    # DMA moves >=4KB contiguous per partition and the PE never does layout.
    W16 = W_b[0].astype(np.float16)  # [D, D]
    # W chunk c as [P, NDC*256]: w[c][p, j*256+d] = W[j*128+p, c*256+d]
    wr = W16.reshape(NDC, P, 4, 256).transpose(2, 1, 0, 3).reshape(4, P, NDC * 256)
    h16 = h.astype(np.float16)
    # hT batch i as [P, NDC*TH]: hT[i][p, c*TH+q] = h[i, q, c*128+p]
    hTr = h16.reshape(B, TH, NDC, P).transpose(0, 3, 2, 1).reshape(B, P, NDC * TH)
    b16 = b.astype(np.float16)
    # bT[i][p, c*TB+k] = b[i, k, c*128+p]
    bTr = b16.reshape(B, TB, NDC, P).transpose(0, 3, 2, 1).reshape(B, P, NDC * TB)
    # bn[i][p, c*D+d] = b[i, c*128+p, d]
    bnr = b16.reshape(B, NKC, P, D).transpose(0, 2, 1, 3).reshape(B, P, NKC * D)
    bbr = np.ascontiguousarray(np.concatenate([bTr, bnr], axis=2))  # [B, P, 2*BB_B]
    ramp_parts = [wr[0], hTr[0], wr[1], wr[2], wr[3]]  # per-core hT below
    ident = np.eye(P, dtype=np.float16)
    in_maps = []
    for c in range(N_CORES):
        sl = slice(c * BPC, (c + 1) * BPC)
        i0 = c * BPC
        ramp_np = np.ascontiguousarray(
            np.concatenate(
                [wr[0], hTr[i0], wr[1], hTr[i0 + 1], wr[2], hTr[i0 + 2],
                 wr[3], hTr[i0 + 3]],
                axis=1,
            )
        )
        in_maps.append(
            {
                "bb": bbr[sl],
                "ramp": ramp_np,
                "ident": ident,
            }
        )
    res = run_bass_kernel_spmd(
        _get_nc(), in_maps, core_ids=list(range(N_CORES)), trace=trace
    )
    out = np.concatenate([res.results[c]["out"] for c in range(N_CORES)], axis=0)
    return out.astype(np.float32), res


def kernel(b, h, W_b):
    out, _ = run(b, h, W_b, trace=False)
    return out
